# revision 16
# baseline (speedup 1.0000x reference)
"""Trainium2 Bass kernel for nn_AGCBlock.

Math: the reference's Sa_GC spatial pool applies log_softmax over a
singleton axis (shape [N, 1, KK]), which is exactly zero, so the pooled
context is exactly zero for every patch.  The channel_add branch then
reduces to a constant vector:

    t    = b1                      (context @ w1.T == 0 exactly)
    tn   = relu(LN(t) * gamma + beta)
    term = w2 @ tn + b2            # [64], independent of x and the patch

and out_p = patches + term.  fold(unfold(x) + term) / fold(unfold(1)) =
x + term (overlap counts cancel; stride 7 < kernel 15 covers every
pixel).  So the whole block is a memory-bound broadcast add:

    out[b, c, h, w] = x[b, c, h, w] + term[c]

(verified vs the jax reference: rel fro err 4.6e-08 in f32).

Distribution: data-parallel over channels -- core i handles channels
[8i, 8i+8), a contiguous zero-copy slice of x.  Each core computes its
8 entries of `term` on device (LayerNorm chain on the vector engine, a
K=1 ones-matmul on the tensor engine to broadcast tn across partitions,
then a row-wise dot with its pre-replicated w2 shard) and streams its
x-shard through SBUF adding term per partition.  Layout per core:
[8, 512, 512] viewed as [128, FREE] with partition p <-> (channel p//16,
row-block p%16); the term repetition is folded into the host-side w2/b2
shard layout (one replicated row per partition), so no on-device
shuffle is needed.

Performance notes (from neuron-profile traces):
- All small parameters are packed into TWO dma_starts (lnp [1,96] =
  b1|gamma|beta, wpack [128,33] = w2rep|b2rep): six separate tiny DMAs
  ahead of the x loads in the HWDGE FIFO cost ~4us of ramp-up.
- Tile sizes (8192, 4096, 4096): 2 MiB leading DMA for transfer
  efficiency, smaller trailing tiles so the final load->add->store
  chain exposes less latency at the window tail.
- I/O precision: x is streamed as fp16 (host casts), the add runs with
  an f32 per-partition bias on the vector engine, the result is stored
  as fp16 (rel fro err vs the f32 reference ~2.9e-4, far inside the
  rel-err gate).  Set KERNEL_IMPL=f32 for a pure-f32 pipeline
  (rel err 3.5e-8, ~1.7x slower: the kernel is pure HBM-bandwidth).
"""

import os
import numpy as np
from contextlib import ExitStack

import concourse.tile as tile
from concourse import bacc, mybir
from concourse.bass_utils import run_bass_kernel_spmd

B, C, H, W = 1, 64, 512, 512
NCORES = 8
CPC = C // NCORES          # 8 channels per core
P = 128                    # SBUF partitions
HH = P // CPC              # 16 row-blocks per channel
FREE = (H // HH) * W       # 32 * 512 = 16384 elements per partition
PLANES = 32
EPS = 1e-5

IMPL = os.environ.get("KERNEL_IMPL", "fp16")
_CFG = {
    "fp16": ((8192, 4096, 4096), np.float16, mybir.dt.float16),
    "f32": ((4096, 4096, 4096, 4096), np.float32, mybir.dt.float32),
}
TILES, NP_DT, MB_DT = _CFG[IMPL]

_nc_cache = []


def _build(tiles=TILES, io_dt=MB_DT):
    f32 = mybir.dt.float32
    nc = bacc.Bacc("TRN2", target_bir_lowering=False, debug=False,
                   num_devices=NCORES)

    x_h = nc.declare_dram_parameter("x", [P, FREE], io_dt, isOutput=False)
    lnp_h = nc.declare_dram_parameter("lnp", [1, 3 * PLANES], f32,
                                      isOutput=False)
    wp_h = nc.declare_dram_parameter("wpack", [P, PLANES + 1], f32,
                                     isOutput=False)
    out_h = nc.declare_dram_parameter("out", [P, FREE], io_dt, isOutput=True)

    nt = len(tiles)
    assert sum(tiles) == FREE
    with tile.TileContext(nc) as tc:
        with ExitStack() as ctx:
            singles = ctx.enter_context(tc.tile_pool(name="singles", bufs=1))
            psum = ctx.enter_context(
                tc.tile_pool(name="psum", bufs=1, space="PSUM"))
            # one SBUF slot per tile (distinct tags, all resident)
            xpool = ctx.enter_context(tc.tile_pool(name="x", bufs=1))

            lnp = singles.tile([1, 3 * PLANES], f32)
            nc.sync.dma_start(lnp[:], lnp_h[:])
            wp = singles.tile([P, PLANES + 1], f32)
            nc.sync.dma_start(wp[:], wp_h[:])
            onesr = singles.tile([1, P], f32)
            nc.vector.memset(onesr[:], 1.0)

            b1r = lnp[:, 0:PLANES]
            gr = lnp[:, PLANES:2 * PLANES]
            ber = lnp[:, 2 * PLANES:3 * PLANES]
            w2s = wp[:, 0:PLANES]
            b2c = wp[:, PLANES:PLANES + 1]

            # ---- LayerNorm(b1) * gamma + beta, relu (partition 0; sc1
            #      holds mu, E[x^2]->var+eps, mu^2->inv, std)
            sc1 = singles.tile([1, 4], f32)
            scr = singles.tile([1, PLANES], f32)
            nc.vector.reduce_sum(sc1[:, 0:1], b1r, axis=mybir.AxisListType.X)
            nc.vector.tensor_mul(scr[:], b1r, b1r)
            nc.vector.reduce_sum(sc1[:, 1:2], scr[:],
                                 axis=mybir.AxisListType.X)
            nc.vector.tensor_scalar_mul(sc1[:, 0:1], sc1[:, 0:1], 1.0 / PLANES)
            nc.vector.tensor_scalar_mul(sc1[:, 1:2], sc1[:, 1:2], 1.0 / PLANES)
            nc.vector.tensor_mul(sc1[:, 2:3], sc1[:, 0:1], sc1[:, 0:1])
            nc.vector.tensor_sub(sc1[:, 1:2], sc1[:, 1:2], sc1[:, 2:3])
            nc.vector.tensor_scalar_add(sc1[:, 1:2], sc1[:, 1:2], EPS)
            nc.scalar.sqrt(sc1[:, 3:4], sc1[:, 1:2])
            nc.vector.reciprocal(sc1[:, 2:3], sc1[:, 3:4])
            nc.vector.tensor_scalar_sub(scr[:], b1r, sc1[:, 0:1])
            nc.vector.tensor_scalar_mul(scr[:], scr[:], sc1[:, 2:3])
            nc.vector.tensor_mul(scr[:], scr[:], gr)
            nc.vector.tensor_add(scr[:], scr[:], ber)
            tnr = singles.tile([1, PLANES], f32)
            nc.vector.tensor_scalar_max(tnr[:], scr[:], 0.0)

            # ---- term[p] = w2rep[p] . tn + b2rep[p]  ([P, 1])
            pb = psum.tile([P, PLANES], f32)
            nc.tensor.matmul(pb[:], onesr[:], tnr[:])
            prod = singles.tile([P, PLANES], f32)
            nc.vector.tensor_mul(prod[:], w2s, pb[:])
            term = singles.tile([P, 1], f32)
            nc.vector.reduce_sum(term[:], prod[:], axis=mybir.AxisListType.X)
            nc.vector.tensor_add(term[:], term[:], b2c)

            # ---- main stream: out = x + term (per-partition f32 bias).
            #      The last tile's add+store is split in half across
            #      VectorE and ScalarE with two half-stores, shortening
            #      the exposed load->add->store chain at the window tail.
            off = 0
            for j, ts in enumerate(tiles):
                sl = slice(off, off + ts)
                off += ts
                t = xpool.tile([P, ts], io_dt, tag=f"x{j}")
                nc.sync.dma_start(t[:], x_h[:, sl])
                if j == nt - 1:
                    hs = ts // 2
                    nc.vector.tensor_scalar_add(t[:, 0:hs], t[:, 0:hs],
                                                term[:])
                    nc.scalar.add(t[:, hs:ts], t[:, hs:ts], term[:])
                    nc.sync.dma_start(out_h[:, sl.start:sl.start + hs],
                                      t[:, 0:hs])
                    nc.sync.dma_start(out_h[:, sl.start + hs:sl.stop],
                                      t[:, hs:ts])
                else:
                    nc.vector.tensor_scalar_add(t[:], t[:], term[:])
                    nc.sync.dma_start(out_h[:, sl], t[:])

    nc.finalize()
    return nc


def make_in_maps(x, b1, gamma, beta, w2, b2):
    x = np.asarray(x, dtype=np.float32)
    b1 = np.asarray(b1, dtype=np.float32).reshape(1, PLANES)
    gamma = np.asarray(gamma, dtype=np.float32).reshape(1, PLANES)
    beta = np.asarray(beta, dtype=np.float32).reshape(1, PLANES)
    w2 = np.asarray(w2, dtype=np.float32).reshape(C, PLANES)
    b2 = np.asarray(b2, dtype=np.float32).reshape(C, 1)
    xs = np.ascontiguousarray(x).reshape(C, H, W).astype(NP_DT, copy=False)
    lnp = np.concatenate([b1, gamma, beta], axis=1)    # [1, 96]
    in_maps = []
    for i in range(NCORES):
        c0 = i * CPC
        wpack = np.concatenate(
            [np.repeat(w2[c0:c0 + CPC], HH, axis=0),
             np.repeat(b2[c0:c0 + CPC], HH, axis=0)], axis=1)  # [P, 33]
        in_maps.append({
            "x": xs[c0:c0 + CPC].reshape(P, FREE),
            "lnp": lnp,
            "wpack": wpack,
        })
    return in_maps


def kernel(x, w_mask, b_mask, w1, b1, gamma, beta, w2, b2):
    if not _nc_cache:
        _nc_cache.append(_build())
    nc = _nc_cache[0]
    in_maps = make_in_maps(x, b1, gamma, beta, w2, b2)
    res = run_bass_kernel_spmd(nc, in_maps, core_ids=list(range(NCORES)))
    out = np.concatenate(
        [res.results[i]["out"].astype(np.float32).reshape(CPC, H, W)
         for i in range(NCORES)],
        axis=0,
    )
    return out.reshape(B, C, H, W)
